# revision 1
# baseline (speedup 1.0000x reference)
"""ContextAttention Trainium2 kernel (8 NeuronCores).

Sharding: core i handles batch b=i//2, sequence half i%2 (2048 rows of N=4096).
All activations live transposed ([C, n] layout) so the contraction dim is on
partitions; per-(b,h) reductions over the full N are completed with a tiny
pairwise bf16 AllReduce between the two half-cores of each batch.

Math (per core, H=12 heads, D=64, C=768, n=2048 local rows):
  qkvT = qkv_w.T^T @ xT  (bf16, f32 psum), m-major weight slabs
  delu(z) = relu(10z) + min(exp(10z), 1)    (2 ACT passes from psum; DVE
            tensor_tensor_reduce fuses the add with the ksum reduce and the
            *v product with the kvd reduce)
  AllReduce [ksum|kvd|lkvd] in bf16; all post-collective scalars + the
  kvd->proj-weight folds run on the otherwise-idle Pool engine so neither
  ACT nor DVE queues stall behind the collective.
  s[h,n] = sum_d q[hd,n]*ksum[hd]  via block one-hot matmul; norm = 1/s
  phase B is chunk-major (512 cols): s -> recip -> one-hot norm bcast ->
  t1 = q*norm (in-place) -> both projections, outputs stream out per chunk.
"""

import numpy as np
import ml_dtypes

import concourse.bass as bass
import concourse.mybir as mybir
import concourse.tile as tile
from concourse import bacc
from concourse.bass_utils import run_bass_kernel_spmd

bf16 = ml_dtypes.bfloat16
dt = mybir.dt
AF = mybir.ActivationFunctionType
OP = mybir.AluOpType

P = 128
NS = 2048          # local sequence rows per core
C = 768
H = 12
D = 64
KT = 6             # C // P     (k tiles / proj tiles)
NM = 18            # 3C // P    (qkv output m-tiles)
NCH = 4            # NS // 512  (matmul free-dim chunks)
FD = 512
EPS = 1e-10
SC = 10.0          # delu parameter
SCALE = D ** -0.5  # 0.125
RG = [[0, 1], [2, 3], [4, 5], [6, 7]]

_CACHE = {}


def _build():
    nc = bacc.Bacc("TRN2", target_bir_lowering=False, debug=False, num_devices=8)

    xT_in = nc.dram_tensor("xT", [C, NS], dt.bfloat16, kind="ExternalInput").ap()
    yT_in = nc.dram_tensor("yT", [C, NS], dt.bfloat16, kind="ExternalInput").ap()
    wq_in = nc.dram_tensor("wqs", [NM * P, KT * P], dt.bfloat16,
                           kind="ExternalInput").ap()
    p1_in = nc.dram_tensor("p1", [C, C], dt.bfloat16, kind="ExternalInput").ap()
    p2_in = nc.dram_tensor("p2", [C, C], dt.bfloat16, kind="ExternalInput").ap()
    b1_in = nc.dram_tensor("b1", [P, KT], dt.float32, kind="ExternalInput").ap()
    b2_in = nc.dram_tensor("b2", [P, KT], dt.float32, kind="ExternalInput").ap()
    oh_in = nc.dram_tensor("oh", [H, C], dt.bfloat16, kind="ExternalInput").ap()
    xo_out = nc.dram_tensor("xo", [C, NS], dt.bfloat16, kind="ExternalOutput").ap()
    yo_out = nc.dram_tensor("yo", [C, NS], dt.bfloat16, kind="ExternalOutput").ap()

    xT3 = xT_in.rearrange("(o p) f -> p o f", p=P)
    yT3 = yT_in.rearrange("(o p) f -> p o f", p=P)
    wq3 = wq_in.rearrange("(m p) f -> p m f", p=P)   # [128, 18, 768] m-slabs
    p13 = p1_in.rearrange("(o p) f -> p o f", p=P)
    p23 = p2_in.rearrange("(o p) f -> p o f", p=P)

    with tile.TileContext(nc) as tc:
        with (
            tc.tile_pool(name="persist", bufs=1) as pp,
            tc.tile_pool(name="scr2", bufs=2) as scr,
            tc.tile_pool(name="scr1", bufs=1) as sc1,
            tc.tile_pool(name="dram", bufs=1, space="DRAM") as dram,
        ):
            ccin = dram.tile([P, 3 * KT], dt.float32)
            ccout = dram.tile([P, 3 * KT], dt.float32)

            qbf = pp.tile([P, KT, NS], dt.bfloat16)
            p1 = pp.tile([P, KT, C], dt.bfloat16)
            p2 = pp.tile([P, KT, C], dt.bfloat16)
            p1f = pp.tile([P, KT, C], dt.bfloat16)
            p2f = pp.tile([P, KT, C], dt.bfloat16)
            red = pp.tile([P, 3 * KT], dt.float32)
            gred = pp.tile([P, 3 * KT], dt.float32)
            ksum = pp.tile([P, KT], dt.float32)
            kvls = pp.tile([P, 2 * KT], dt.float32)
            lhsT3 = pp.tile([P, KT, H], dt.bfloat16)
            oh_sb = pp.tile([H, C], dt.bfloat16)
            b1_sb = pp.tile([P, KT], dt.float32)
            b2_sb = pp.tile([P, KT], dt.float32)
            snorm = pp.tile([H, NS], dt.float32)
            snorm_bf = pp.tile([H, NS], dt.bfloat16)

            pa = ctx_pa = tc.tile_pool(name="phA", bufs=1)
            pa = ctx_pa.__enter__()
            xT = pa.tile([P, KT, NS], dt.bfloat16)
            yT = pa.tile([P, KT, NS], dt.bfloat16)
            wq = pa.tile([P, NM, KT * P], dt.bfloat16)

            # ---- input DMAs, all on the sync queue in priority order:
            # xT in 24 chunk-DMAs (spreads across DMA rings) and the k/v
            # weight slabs first — they gate the first matmuls — then
            # everything else.
            for kk in range(KT):
                for ch in range(NCH):
                    cs = slice(ch * FD, (ch + 1) * FD)
                    nc.sync.dma_start(xT[:, kk, cs], xT3[:, kk, cs])
            for m in (6, 12, 7, 13, 8, 14, 9, 15, 10, 16, 11, 17):
                nc.sync.dma_start(wq[:, m, :], wq3[:, m, :])
            for kk in range(KT):
                nc.sync.dma_start(yT[:, kk, :], yT3[:, kk, :])
            for m in range(KT):
                nc.sync.dma_start(wq[:, m, :], wq3[:, m, :])
            for kk in range(KT):
                nc.sync.dma_start(p1[:, kk, :], p13[:, kk, :])
                nc.sync.dma_start(p2[:, kk, :], p23[:, kk, :])
            nc.sync.dma_start(oh_sb[:], oh_in[:])
            nc.sync.dma_start(b1_sb[:], b1_in[:])
            nc.sync.dma_start(b2_sb[:], b2_in[:])

            nc.vector.memset(lhsT3[:], 0.0)

            with tc.tile_pool(name="psA", bufs=1, space="PSUM") as psA:

                def mm_tile(m, tag):
                    """qkv output m-tile -> [128, NS] psum (f32)."""
                    ps = psA.tile([P, NS], dt.float32, tag=tag)
                    for kk in range(KT):
                        for ch in range(NCH):
                            cs = slice(ch * FD, (ch + 1) * FD)
                            nc.tensor.matmul(
                                ps[:, cs],
                                wq[:, m, kk * P:(kk + 1) * P],
                                xT[:, kk, cs],
                                start=(kk == 0),
                                stop=(kk == KT - 1),
                            )
                    return ps

                # ---------- k/v loop: reductions for ksum/kvd/lkvd
                for j in range(KT):
                    ps_k = mm_tile(6 + j, "pk")
                    r = scr.tile([P, NS], dt.bfloat16, tag="r")
                    e = scr.tile([P, NS], dt.bfloat16, tag="e")
                    nc.scalar.activation(r[:], ps_k[:], AF.Relu, scale=SC)
                    nc.scalar.activation(e[:], ps_k[:], AF.Exp, scale=SC)

                    ps_v = mm_tile(12 + j, "pv")
                    v32 = scr.tile([P, NS], dt.bfloat16, tag="v")
                    nc.scalar.copy(v32[:], ps_v[:])

                    ko = sc1.tile([P, NS], dt.bfloat16, tag="ko")
                    em = sc1.tile([P, NS], dt.bfloat16, tag="em")
                    prod = sc1.tile([P, NS], dt.bfloat16, tag="prod")
                    nc.vector.tensor_scalar_min(em[:], e[:], 1.0)
                    nc.vector.tensor_tensor(ko[:], r[:], em[:], OP.add)
                    nc.vector.reduce_sum(red[:, j:j + 1], ko[:],
                                         axis=mybir.AxisListType.X)
                    nc.vector.tensor_tensor(prod[:], ko[:], v32[:], OP.mult)
                    nc.vector.reduce_sum(red[:, KT + j:KT + j + 1], prod[:],
                                         axis=mybir.AxisListType.X)

                    r2 = sc1.tile([P, NS], dt.bfloat16, tag="r2")
                    e2 = sc1.tile([P, NS], dt.bfloat16, tag="e2")
                    lko = sc1.tile([P, NS], dt.bfloat16, tag="lko")
                    em2 = sc1.tile([P, NS], dt.bfloat16, tag="em2")
                    nc.scalar.activation(r2[:], yT[:, j, :], AF.Relu, scale=SC)
                    nc.scalar.activation(e2[:], yT[:, j, :], AF.Exp, scale=SC)
                    nc.vector.tensor_scalar_min(em2[:], e2[:], 1.0)
                    nc.vector.tensor_tensor(lko[:], r2[:], em2[:], OP.add)
                    nc.vector.tensor_tensor(prod[:], lko[:], v32[:], OP.mult)
                    nc.vector.reduce_sum(red[:, 2 * KT + j:2 * KT + j + 1],
                                         prod[:], axis=mybir.AxisListType.X)

                # ---------- pairwise AllReduce of [ksum | kvd | lkvd]
                nc.gpsimd.dma_start(ccin[:], red[:])
                nc.gpsimd.collective_compute(
                    "AllReduce", OP.add, replica_groups=RG,
                    ins=[ccin.opt()], outs=[ccout.opt()],
                )
                nc.gpsimd.dma_start(gred[:], ccout[:])

                # ---------- q loop (collective hides under it)
                for j in range(KT):
                    ps_q = mm_tile(j, "pk" if j % 2 == 0 else "pv")
                    r = scr.tile([P, NS], dt.bfloat16, tag="r")
                    e = scr.tile([P, NS], dt.bfloat16, tag="e")
                    em = sc1.tile([P, NS], dt.bfloat16, tag="em")
                    nc.scalar.activation(r[:], ps_q[:], AF.Relu, scale=SC)
                    nc.scalar.activation(e[:], ps_q[:], AF.Exp, scale=SC)
                    nc.vector.tensor_scalar_min(em[:], e[:], 1.0)
                    nc.vector.tensor_tensor(qbf[:, j, :], r[:], em[:], OP.add)

                # post-collective scalar chain, emitted AFTER the q-loop ops
                # so the q-tile delu epilogues never queue behind the
                # collective (the baseline's seam stall)
                nc.vector.tensor_scalar_add(ksum[:], gred[:, 0:KT], EPS)
                nc.vector.tensor_scalar_mul(kvls[:], gred[:, KT:3 * KT], SCALE)
                for j in range(KT):
                    nc.vector.tensor_copy(lhsT3[0:64, j, 2 * j:2 * j + 1],
                                          ksum[0:64, j:j + 1])
                    nc.vector.tensor_copy(lhsT3[64:128, j, 2 * j + 1:2 * j + 2],
                                          ksum[64:128, j:j + 1])
                # kvd/lkvd folds into the projection weights (ACT, after the
                # q-loop ACT ops; interleaved p1/p2 so early kk land first)
                for kk in range(KT):
                    nc.scalar.mul(p1f[:, kk, :], p1[:, kk, :],
                                  kvls[:, kk:kk + 1])
                    nc.scalar.mul(p2f[:, kk, :], p2[:, kk, :],
                                  kvls[:, KT + kk:KT + kk + 1])

            ctx_pa.__exit__(None, None, None)

            # ---------------- phase B: norm, t1, projections (chunk-major)
            with (
                tc.tile_pool(name="phB", bufs=1) as pb,
                tc.tile_pool(name="psS", bufs=2, space="PSUM") as psS,
                tc.tile_pool(name="psB", bufs=3, space="PSUM") as psB,
                tc.tile_pool(name="psO", bufs=3, space="PSUM") as psO,
                tc.tile_pool(name="outp", bufs=4) as outp,
            ):
                t1 = pb.tile([P, KT, NS], dt.bfloat16)
                for ch in range(NCH):
                    cs = slice(ch * FD, (ch + 1) * FD)
                    ps_s = psS.tile([H, FD], dt.float32, tag="s")
                    for j in range(KT):
                        nc.tensor.matmul(
                            ps_s[:],
                            lhsT3[:, j, :],
                            qbf[:, j, cs],
                            start=(j == 0),
                            stop=(j == KT - 1),
                        )
                    nc.vector.reciprocal(snorm[:, cs], ps_s[:])
                    nc.scalar.copy(snorm_bf[:, cs], snorm[:, cs])

                for ch in range(NCH):
                    cs = slice(ch * FD, (ch + 1) * FD)
                    for j in range(KT):
                        ps_bc = psB.tile([P, FD], dt.float32, tag="bc")
                        nc.tensor.matmul(
                            ps_bc[:],
                            oh_sb[:, j * P:(j + 1) * P],
                            snorm_bf[:, cs],
                            start=True, stop=True,
                        )
                        nc.vector.tensor_tensor(t1[:, j, cs], qbf[:, j, cs],
                                                ps_bc[:], OP.mult)
                    for mo in range(KT):
                        for wf, bias, dst in ((p1f, b1_sb, xo_out),
                                              (p2f, b2_sb, yo_out)):
                            ps_o = psO.tile([P, FD], dt.float32, tag="o")
                            for kk in range(KT):
                                nc.tensor.matmul(
                                    ps_o[:],
                                    wf[:, kk, mo * P:(mo + 1) * P],
                                    t1[:, kk, cs],
                                    start=(kk == 0),
                                    stop=(kk == KT - 1),
                                )
                            osb = outp.tile([P, FD], dt.bfloat16, tag="osb")
                            nc.scalar.activation(osb[:], ps_o[:], AF.Identity,
                                                 bias=bias[:, mo:mo + 1],
                                                 scale=1.0)
                            nc.sync.dma_start(dst[mo * P:(mo + 1) * P, cs],
                                              osb[:])

    nc.compile()
    return nc


def _get_nc():
    if "nc" not in _CACHE:
        _CACHE["nc"] = _build()
    return _CACHE["nc"]


def _make_in_maps(x, y, qkv_w, proj1_w, proj1_b, proj2_w, proj2_b):
    wqT = np.ascontiguousarray(np.asarray(qkv_w).T).astype(bf16)  # [C, 3C]
    # m-major slabs: row m*128+p, col kk*128+mc  ==  qkv_w[m*128+mc, kk*128+p]
    wqs = np.ascontiguousarray(
        wqT.reshape(KT, P, NM, P).transpose(2, 1, 0, 3).reshape(NM * P, KT * P))
    p1_np = np.ascontiguousarray(np.asarray(proj1_w).T).astype(bf16)
    p2_np = np.ascontiguousarray(np.asarray(proj2_w).T).astype(bf16)
    b1_np = np.ascontiguousarray(np.asarray(proj1_b, np.float32).reshape(KT, P).T)
    b2_np = np.ascontiguousarray(np.asarray(proj2_b, np.float32).reshape(KT, P).T)
    oh_np = np.zeros((H, C), bf16)
    for j in range(KT):
        oh_np[2 * j, j * P:j * P + 64] = 1
        oh_np[2 * j + 1, j * P + 64:(j + 1) * P] = 1
    in_maps = []
    for core in range(8):
        b_, h_ = core // 2, core % 2
        sl = slice(h_ * NS, (h_ + 1) * NS)
        xT = np.ascontiguousarray(np.asarray(x)[b_, sl].T).astype(bf16)
        yT = np.ascontiguousarray(np.asarray(y)[b_, sl].T).astype(bf16)
        in_maps.append({"xT": xT, "yT": yT, "wqs": wqs, "p1": p1_np,
                        "p2": p2_np, "b1": b1_np, "b2": b2_np, "oh": oh_np})
    return in_maps


def _unshard(results, B, N):
    xo = np.empty((B, N, C), np.float32)
    yo = np.empty((B, N, C), np.float32)
    for core in range(8):
        b_, h_ = core // 2, core % 2
        sl = slice(h_ * NS, (h_ + 1) * NS)
        xo[b_, sl] = results[core]["xo"].astype(np.float32).T
        yo[b_, sl] = results[core]["yo"].astype(np.float32).T
    return xo, yo


def kernel(x, y, qkv_w, proj1_w, proj1_b, proj2_w, proj2_b):
    nc = _get_nc()
    in_maps = _make_in_maps(x, y, qkv_w, proj1_w, proj1_b, proj2_w, proj2_b)
    res = run_bass_kernel_spmd(nc, in_maps, list(range(8)))
    return _unshard(res.results, np.asarray(x).shape[0], np.asarray(x).shape[1])



# revision 15
# speedup vs baseline: 1.2860x; 1.2860x over previous
"""ContextAttention Trainium2 kernel (8 NeuronCores), v3.

Sharding: core i handles batch b=i//2, sequence half i%2 (2048 rows of N=4096).
Activations live transposed ([C, n]) so the contraction dim is on partitions;
per-(b,h) reductions over the full N complete with a pairwise f32 AllReduce
between the two half-cores of each batch.

v3 structure (vs v1 baseline at ~285us):
  - Inputs host-packed partition-major, loaded with ~14 large DMAs in
    consumption order (each DMA_DIRECT2D costs ~0.7us serial issue time).
  - k/v loop: v stays in PSUM until Pool copies it (tensor_copy on the idle
    Pool engine); kvd/lkvd reductions use the HW-validated custom DVE op
    affine_mul_reduce (out=(ko*SCALE)*v, accum=sum) - one DVE pass instead
    of mult+reduce. (The generic stt/ttr bass ops crash the DVE ucode:
    NRT_EXEC_UNIT_UNRECOVERABLE - their uop-table rows never load.)
  - kvd/lkvd are not folded into projection weights (12 ACT passes on the
    post-collective critical path in v1); they fold into the t1 product via
    affine_mul_reduce scale slots: t1x = (qbf*kvd_col)*norm_bcast.
  - s-matmuls run right after the q loop; collective DMAs on Pool.
"""

import numpy as np
import ml_dtypes

import concourse.bass as bass
import concourse.mybir as mybir
import concourse.tile as tile
from concourse import bacc
from concourse.bass_utils import run_bass_kernel_spmd

bf16 = ml_dtypes.bfloat16
dt = mybir.dt
AF = mybir.ActivationFunctionType
OP = mybir.AluOpType

P = 128
NS = 2048          # local sequence rows per core
C = 768
H = 12
D = 64
KT = 6             # C // P     (k tiles / proj tiles)
NCH = 4            # NS // 512  (matmul free-dim chunks)
FD = 512
EPS = 1e-10
SC = 10.0          # delu parameter
SCALE = D ** -0.5  # 0.125
RG = [[0, 1], [2, 3], [4, 5], [6, 7]]

# wq slot order = consumption order: k/v interleaved, then q
MORDER = [6, 12, 7, 13, 8, 14, 9, 15, 10, 16, 11, 17, 0, 1, 2, 3, 4, 5]
SLOT = {m: i for i, m in enumerate(MORDER)}

_CACHE = {}


def _build():
    nc = bacc.Bacc("TRN2", target_bir_lowering=False, debug=False, num_devices=8)

    xT_in = nc.dram_tensor("xT", [P, KT, NS], dt.bfloat16, kind="ExternalInput").ap()
    yT_in = nc.dram_tensor("yT", [P, KT, NS], dt.bfloat16, kind="ExternalInput").ap()
    w6_in = nc.dram_tensor("w6", [P, 1, C], dt.bfloat16, kind="ExternalInput").ap()
    w12_in = nc.dram_tensor("w12", [P, 1, C], dt.bfloat16, kind="ExternalInput").ap()
    wkv1_in = nc.dram_tensor("wkv1", [P, 4, C], dt.bfloat16, kind="ExternalInput").ap()
    wkv2_in = nc.dram_tensor("wkv2", [P, 6, C], dt.bfloat16, kind="ExternalInput").ap()
    wq0_in = nc.dram_tensor("wq0", [P, KT, C], dt.bfloat16, kind="ExternalInput").ap()
    p1_in = nc.dram_tensor("p1", [P, KT, C], dt.bfloat16, kind="ExternalInput").ap()
    p2_in = nc.dram_tensor("p2", [P, KT, C], dt.bfloat16, kind="ExternalInput").ap()
    b1_in = nc.dram_tensor("b1", [P, KT], dt.float32, kind="ExternalInput").ap()
    b2_in = nc.dram_tensor("b2", [P, KT], dt.float32, kind="ExternalInput").ap()
    oh_in = nc.dram_tensor("oh", [H, C], dt.bfloat16, kind="ExternalInput").ap()
    xo_out = nc.dram_tensor("xo", [C, NS], dt.bfloat16, kind="ExternalOutput").ap()
    yo_out = nc.dram_tensor("yo", [C, NS], dt.bfloat16, kind="ExternalOutput").ap()

    with tile.TileContext(nc) as tc:
        with (
            tc.tile_pool(name="persist", bufs=1) as pp,
            tc.tile_pool(name="dram", bufs=1, space="DRAM") as dram,
        ):
            ccin = dram.tile([P, 3 * KT], dt.float32)
            ccout = dram.tile([P, 3 * KT], dt.float32)

            qbf = pp.tile([P, KT, NS], dt.bfloat16)
            p1 = pp.tile([P, KT, C], dt.bfloat16)
            p2 = pp.tile([P, KT, C], dt.bfloat16)
            red = pp.tile([P, 3 * KT], dt.float32)
            gred = pp.tile([P, 3 * KT], dt.float32)
            ksum = pp.tile([P, KT], dt.float32)
            lhsT3 = pp.tile([P, KT, H], dt.bfloat16)
            oh_sb = pp.tile([H, C], dt.bfloat16)
            b1_sb = pp.tile([P, KT], dt.float32)
            b2_sb = pp.tile([P, KT], dt.float32)
            snorm = pp.tile([H, NS], dt.float32)
            snorm_bf = pp.tile([H, NS], dt.bfloat16)

            ctx_pa = tc.tile_pool(name="phA", bufs=1)
            pa = ctx_pa.__enter__()
            xT = pa.tile([P, KT, NS], dt.bfloat16)
            yT = pa.tile([P, KT, NS], dt.bfloat16)
            wq = pa.tile([P, 18, C], dt.bfloat16)

            ctx_sc = tc.tile_pool(name="scr2", bufs=2)
            scr = ctx_sc.__enter__()
            ctx_s1 = tc.tile_pool(name="scr1", bufs=1)
            sc1 = ctx_s1.__enter__()

            # ---- input DMAs: consumption order on the sync HWDGE queue.
            nc.sync.dma_start(wq[:, 0:1, :], w6_in[:])
            nc.sync.dma_start(wq[:, 1:2, :], w12_in[:])
            nc.sync.dma_start(xT[:, 0:3, :], xT_in[:, 0:3, :])
            nc.sync.dma_start(yT[:, 0:2, :], yT_in[:, 0:2, :])
            nc.sync.dma_start(xT[:, 3:6, :], xT_in[:, 3:6, :])
            nc.sync.dma_start(wq[:, 2:6, :], wkv1_in[:])
            nc.sync.dma_start(yT[:, 2:6, :], yT_in[:, 2:6, :])
            nc.sync.dma_start(wq[:, 6:12, :], wkv2_in[:])
            nc.sync.dma_start(wq[:, 12:18, :], wq0_in[:])
            nc.sync.dma_start(oh_sb[:], oh_in[:])
            nc.sync.dma_start(p1[:], p1_in[:])
            nc.sync.dma_start(p2[:], p2_in[:])
            nc.sync.dma_start(b1_sb[:], b1_in[:])
            nc.sync.dma_start(b2_sb[:], b2_in[:])

            nc.vector.memset(lhsT3[:], 0.0)

            with tc.tile_pool(name="psA", bufs=1, space="PSUM") as psA:

                def mm_tile(m, tag):
                    """qkv output m-tile -> [128, NS] psum (f32)."""
                    ps = psA.tile([P, NS], dt.float32, tag=tag)
                    s = SLOT[m]
                    for kk in range(KT):
                        for ch in range(NCH):
                            cs = slice(ch * FD, (ch + 1) * FD)
                            nc.tensor.matmul(
                                ps[:, cs],
                                wq[:, s, kk * P:(kk + 1) * P],
                                xT[:, kk, cs],
                                start=(kk == 0),
                                stop=(kk == KT - 1),
                            )
                    return ps

                # ---------- k/v loop: ksum/kvd/lkvd reductions
                for j in range(KT):
                    ps_k = mm_tile(6 + j, "pk")
                    # lk path first on ACT: only needs yT (DMA), so it can
                    # run while the first matmuls are still accumulating.
                    e2 = scr.tile([P, NS], dt.bfloat16, tag="e2")
                    r2 = scr.tile([P, NS], dt.bfloat16, tag="r2")
                    nc.scalar.activation(e2[:], yT[:, j, :], AF.Exp, scale=SC)
                    nc.scalar.activation(r2[:], yT[:, j, :], AF.Relu, scale=SC)
                    e = scr.tile([P, NS], dt.bfloat16, tag="e")
                    r = scr.tile([P, NS], dt.bfloat16, tag="r")
                    nc.scalar.activation(e[:], ps_k[:], AF.Exp, scale=SC)
                    nc.scalar.activation(r[:], ps_k[:], AF.Relu, scale=SC)

                    em = scr.tile([P, NS], dt.bfloat16, tag="em")
                    lko = scr.tile([P, NS], dt.bfloat16, tag="lko")
                    ko = scr.tile([P, NS], dt.bfloat16, tag="ko")
                    nc.vector.tensor_scalar_min(em[:], e2[:], 1.0)
                    nc.vector.tensor_tensor(lko[:], em[:], r2[:], OP.add)
                    nc.vector.tensor_scalar_min(em[:], e[:], 1.0)
                    nc.vector.tensor_tensor(ko[:], em[:], r[:], OP.add)
                    nc.vector.reduce_sum(red[:, j:j + 1], ko[:],
                                         axis=mybir.AxisListType.X)

                    ps_v = mm_tile(12 + j, "pv")
                    v32 = scr.tile([P, NS], dt.bfloat16, tag="v32")
                    nc.scalar.copy(v32[:], ps_v[:])
                    # kvd_j = sum((ko*SCALE)*v); lkvd_j likewise - one fused
                    # DVE op each (affine_mul_reduce, HW-validated table op).
                    junk = sc1.tile([P, NS], dt.bfloat16, tag="junk")
                    junk2 = sc1.tile([P, NS], dt.bfloat16, tag="junk2")
                    nc.vector.affine_mul_reduce(
                        out=junk[:], accum_out=red[:, KT + j:KT + j + 1],
                        in0=ko[:], in1=v32[:], scale=SCALE, bias=0.0)
                    nc.vector.affine_mul_reduce(
                        out=junk2[:], accum_out=red[:, 2 * KT + j:2 * KT + j + 1],
                        in0=lko[:], in1=v32[:], scale=SCALE, bias=0.0)

                # ------ pairwise AllReduce of [ksum | kvd | lkvd] on the
                # Pool queue; hides under the q loop.
                nc.gpsimd.dma_start(ccin[:], red[:])
                nc.gpsimd.collective_compute(
                    "AllReduce", OP.add, replica_groups=RG,
                    ins=[ccin.opt()], outs=[ccout.opt()],
                )
                nc.gpsimd.dma_start(gred[:], ccout[:])

                # ---------- q loop (collective hides under it)
                for j in range(KT):
                    ps_q = mm_tile(j, "pk" if j % 2 == 0 else "pv")
                    e = scr.tile([P, NS], dt.bfloat16, tag="e")
                    r = scr.tile([P, NS], dt.bfloat16, tag="r")
                    nc.scalar.activation(e[:], ps_q[:], AF.Exp, scale=SC)
                    nc.scalar.activation(r[:], ps_q[:], AF.Relu, scale=SC)
                    em = scr.tile([P, NS], dt.bfloat16, tag="em")
                    nc.vector.tensor_scalar_min(em[:], e[:], 1.0)
                    nc.vector.tensor_tensor(qbf[:, j, :], em[:], r[:], OP.add)

                # lhsT3 = block one-hot filled with ksum+EPS; emitted after
                # the q epilogues so the gred wait can't head-of-line block
                # the DVE queue (only the s-matmuls need lhsT3).
                nc.vector.tensor_scalar_add(ksum[:], gred[:, 0:KT], EPS)
                for j in range(KT):
                    nc.vector.tensor_copy(lhsT3[0:64, j, 2 * j:2 * j + 1],
                                          ksum[0:64, j:j + 1])
                    nc.vector.tensor_copy(lhsT3[64:128, j, 2 * j + 1:2 * j + 2],
                                          ksum[64:128, j:j + 1])

            ctx_s1.__exit__(None, None, None)
            ctx_sc.__exit__(None, None, None)

            # ---------------- phase B: s, norm, t1x/t1y, projections
            with (
                tc.tile_pool(name="phB", bufs=2) as pb,
                tc.tile_pool(name="psS", bufs=2, space="PSUM") as psS,
                tc.tile_pool(name="psB", bufs=3, space="PSUM") as psB,
                tc.tile_pool(name="psO", bufs=3, space="PSUM") as psO,
                tc.tile_pool(name="outp", bufs=4) as outp,
            ):
                for ch in range(NCH):
                    cs = slice(ch * FD, (ch + 1) * FD)
                    ps_s = psS.tile([H, FD], dt.float32, tag="s")
                    for j in range(KT):
                        nc.tensor.matmul(
                            ps_s[:],
                            lhsT3[:, j, :],
                            qbf[:, j, cs],
                            start=(j == 0),
                            stop=(j == KT - 1),
                        )
                    nc.vector.reciprocal(snorm[:, cs], ps_s[:])
                    nc.scalar.copy(snorm_bf[:, cs], snorm[:, cs])

                for ch in range(NCH):
                    cs = slice(ch * FD, (ch + 1) * FD)
                    t1x = pb.tile([P, KT, FD], dt.bfloat16, tag="t1x")
                    t1y = pb.tile([P, KT, FD], dt.bfloat16, tag="t1y")
                    for j in range(KT):
                        ps_bc = psB.tile([P, FD], dt.float32, tag="bc")
                        nc.tensor.matmul(
                            ps_bc[:],
                            oh_sb[:, j * P:(j + 1) * P],
                            snorm_bf[:, cs],
                            start=True, stop=True,
                        )
                        bcc = pb.tile([P, FD], dt.bfloat16, tag="bcc")
                        nc.scalar.copy(bcc[:], ps_bc[:])
                        # t1x = (qbf*kvd_col)*norm ; t1y likewise with lkvd
                        ja = pb.tile([P, 1], dt.float32, tag="ja")
                        jb = pb.tile([P, 1], dt.float32, tag="jb")
                        nc.vector.affine_mul_reduce(
                            out=t1x[:, j, :], accum_out=ja[:],
                            in0=qbf[:, j, cs], in1=bcc[:],
                            scale=gred[:, KT + j:KT + j + 1], bias=0.0)
                        nc.vector.affine_mul_reduce(
                            out=t1y[:, j, :], accum_out=jb[:],
                            in0=qbf[:, j, cs], in1=bcc[:],
                            scale=gred[:, 2 * KT + j:2 * KT + j + 1], bias=0.0)
                    for mo in range(KT):
                        for wf, bias, t1s, dst in ((p1, b1_sb, t1x, xo_out),
                                                   (p2, b2_sb, t1y, yo_out)):
                            ps_o = psO.tile([P, FD], dt.float32, tag="o")
                            for kk in range(KT):
                                nc.tensor.matmul(
                                    ps_o[:],
                                    wf[:, kk, mo * P:(mo + 1) * P],
                                    t1s[:, kk, :],
                                    start=(kk == 0),
                                    stop=(kk == KT - 1),
                                )
                            osb = outp.tile([P, FD], dt.bfloat16, tag="osb")
                            nc.scalar.activation(osb[:], ps_o[:], AF.Identity,
                                                 bias=bias[:, mo:mo + 1],
                                                 scale=1.0)
                            nc.sync.dma_start(dst[mo * P:(mo + 1) * P, cs],
                                              osb[:])

            ctx_pa.__exit__(None, None, None)

    nc.compile()
    return nc


def _get_nc():
    if "nc" not in _CACHE:
        _CACHE["nc"] = _build()
    return _CACHE["nc"]


def _make_in_maps(x, y, qkv_w, proj1_w, proj1_b, proj2_w, proj2_b):
    wqT = np.ascontiguousarray(np.asarray(qkv_w).T).astype(bf16)  # [C, 3C]
    # [p, m, kk*128+mc] with m in slot (consumption) order
    w4 = wqT.reshape(KT, P, 18, P).transpose(1, 2, 0, 3).reshape(P, 18, C)
    ws = np.ascontiguousarray(w4[:, MORDER])
    w6 = np.ascontiguousarray(ws[:, 0:1])
    w12 = np.ascontiguousarray(ws[:, 1:2])
    wkv1 = np.ascontiguousarray(ws[:, 2:6])
    wkv2 = np.ascontiguousarray(ws[:, 6:12])
    wq0 = np.ascontiguousarray(ws[:, 12:18])

    def pmajor(wT):  # [C, C] -> [p, kk, mo*128+mc]
        a = np.asarray(wT).astype(bf16)
        return np.ascontiguousarray(a.reshape(KT, P, C).transpose(1, 0, 2))

    p1_np = pmajor(np.asarray(proj1_w).T)
    p2_np = pmajor(np.asarray(proj2_w).T)
    b1_np = np.ascontiguousarray(np.asarray(proj1_b, np.float32).reshape(KT, P).T)
    b2_np = np.ascontiguousarray(np.asarray(proj2_b, np.float32).reshape(KT, P).T)
    oh_np = np.zeros((H, C), bf16)
    for j in range(KT):
        oh_np[2 * j, j * P:j * P + 64] = 1
        oh_np[2 * j + 1, j * P + 64:(j + 1) * P] = 1
    in_maps = []
    for core in range(8):
        b_, h_ = core // 2, core % 2
        sl = slice(h_ * NS, (h_ + 1) * NS)
        xT = np.asarray(x)[b_, sl].T.astype(bf16)      # [C, NS]
        yT = np.asarray(y)[b_, sl].T.astype(bf16)
        xTp = np.ascontiguousarray(xT.reshape(KT, P, NS).transpose(1, 0, 2))
        yTp = np.ascontiguousarray(yT.reshape(KT, P, NS).transpose(1, 0, 2))
        in_maps.append({"xT": xTp, "yT": yTp, "w6": w6, "w12": w12,
                        "wkv1": wkv1, "wkv2": wkv2, "wq0": wq0,
                        "p1": p1_np, "p2": p2_np, "b1": b1_np, "b2": b2_np,
                        "oh": oh_np})
    return in_maps


def _unshard(results, B, N):
    xo = np.empty((B, N, C), np.float32)
    yo = np.empty((B, N, C), np.float32)
    for core in range(8):
        b_, h_ = core // 2, core % 2
        sl = slice(h_ * NS, (h_ + 1) * NS)
        xo[b_, sl] = results[core]["xo"].astype(np.float32).T
        yo[b_, sl] = results[core]["yo"].astype(np.float32).T
    return xo, yo


def kernel(x, y, qkv_w, proj1_w, proj1_b, proj2_w, proj2_b):
    nc = _get_nc()
    in_maps = _make_in_maps(x, y, qkv_w, proj1_w, proj1_b, proj2_w, proj2_b)
    res = run_bass_kernel_spmd(nc, in_maps, list(range(8)))
    return _unshard(res.results, np.asarray(x).shape[0], np.asarray(x).shape[1])
